# revision 19
# baseline (speedup 1.0000x reference)
"""Trainium2 Bass kernel for nn_CrossSemanticAttentionModule0 (cross-modal attention).

Sharding: 8 cores = (batch b in {0,1}) x (query/pixel slab s in {0..3}; 16 H-rows
= 1024 pixels each). Each core: conv+BN+PReLU for its slab (with halo), q/k/v
projections, AllGathers K (bf16, with a ones row for the softmax bias trick) and
V^T (fp8 e4m3, x8 scaled) across its 4-core batch group, then two-pass softmax
cross-attention for its query rows over the full key axis, up-projections +
residuals. All matmuls in bf16 (fp32 PSUM accumulate).
"""

import numpy as np
import functools

import ml_dtypes

import concourse.bass as bass
import concourse.mybir as mybir
import concourse.tile as tile
import concourse.bacc as bacc
from concourse.bass_utils import run_bass_kernel_spmd

B, CIN, H, W = 2, 512, 64, 64
CD, CQ = 256, 32
N = H * W                 # 4096 pixels
SLAB_ROWS = 16            # H rows per core
SLAB = SLAB_ROWS * W      # 1024 pixels per core
HR = SLAB_ROWS + 2        # halo rows
WP = W + 2                # padded width
N_CORES = 8
MODS = ("rgb", "dsm")
F32 = mybir.dt.float32
BF16 = mybir.dt.bfloat16
FP8 = mybir.dt.float8e4
AF = mybir.ActivationFunctionType
ALU = mybir.AluOpType
AX = mybir.AxisListType
RG = [[0, 1, 2, 3], [4, 5, 6, 7]]
NKT = N // 128            # 32 key tiles of 128
VSCALE = 1.0              # V carried unscaled (bf16 wire)
MARGIN = 72.0             # softmax max-bound margin over stride-4 subsample


def _build(dbg=False):
    nc = bacc.Bacc("TRN2", target_bir_lowering=False, debug=False,
                   num_devices=N_CORES)
    DBG = {}
    def dbg_out(name, shape, dt):
        if dbg:
            DBG[name] = nc.dram_tensor(name, shape, dt,
                                       kind="ExternalOutput").ap()
    dbg_out("dbg_conv_rgb", [128, 2, SLAB], BF16)
    dbg_out("dbg_qk_dsm", [64, SLAB], BF16)
    dbg_out("dbg_mstack0", [128, 8], F32)
    dbg_out("dbg_QS2_0", [128, SLAB], BF16)
    dbg_out("dbg_KS2_0", [128, N], BF16)
    dbg_out("dbg_vfull0", [128, NKT, CD], BF16)
    dbg_out("dbg_lnl00", [1, 512], F32)
    dbg_out("dbg_recg00", [1, 512], BF16)
    dbg_out("dbg_ocp00", [128, 512], BF16)
    dbg_out("dbg_osb0", [128, 2, SLAB], BF16)
    dbg_out("dbg_K4_0", [128, SLAB], BF16)
    dbg_out("dbg_q4_0", [128, SLAB], BF16)
    dbg_out("dbg_mt00", [128, 4], F32)

    D = {}
    def din(name, shape, dt=F32):
        D[name] = nc.dram_tensor(name, shape, dt, kind="ExternalInput").ap()
    for m in MODS:
        din(f"xs_{m}", [128, 4, HR, WP], BF16)
        din(f"cw_{m}", [9, 4, 128, CD], BF16)
        din(f"bna_{m}", [128, 2])
        din(f"bnb_{m}", [128, 2])
        din(f"alpha_{m}", [128, 1])
        din(f"qkw_{m}", [2, 128, 64], BF16)
        din(f"qkb_{m}", [64, 1])
        din(f"vw_{m}", [2, 128, CD], BF16)
        din(f"upw_{m}", [2, 128, CIN], BF16)
        din(f"upb_{m}", [128, 4])
        din(f"gvb_{m}", [128, 2])
        din(f"gam_{m}", [1, 1])
    din("negI", [128, 128])
    OUT = {m: nc.dram_tensor(f"out_{m}", [CIN, SLAB], F32,
                             kind="ExternalOutput").ap() for m in MODS}

    with tile.TileContext(nc) as tc:
        with (
            tc.tile_pool(name="const", bufs=1) as cpool,
            tc.tile_pool(name="cw", bufs=3) as cwpool,
            tc.tile_pool(name="big", bufs=1) as bpool,
            tc.tile_pool(name="pair", bufs=2) as prpool,
            tc.tile_pool(name="pt", bufs=3) as ptpool,
            tc.tile_pool(name="eps", bufs=2) as epool,
            tc.tile_pool(name="dram", bufs=1, space="DRAM") as dpool,
        ):
            # ---- constants / weights to SBUF (loaded just-in-time) ----
            sb = {}
            def load_const(m, names):
                specs = {
                    "xs": ([128, 4, HR, WP], BF16), "bna": ([128, 2], F32),
                    "bnb": ([128, 2], F32), "alpha": ([128, 1], F32),
                    "qkw": ([128, 2, 64], BF16), "qkb": ([64, 1], F32),
                    "vw": ([128, 2, CD], BF16), "upw": ([128, 2, CIN], BF16),
                    "upb": ([128, 4], F32), "gvb": ([128, 2], F32),
                    "gam": ([1, 1], F32),
                }
                for base in names:
                    nm = f"{base}_{m}"
                    shp, dt = specs[base]
                    t = cpool.tile(shp, dt, tag=nm, name=nm)
                    src = D[nm]
                    if base in ("qkw", "vw", "upw"):
                        src = src.rearrange("k p f -> p k f", p=128)
                    if base == "xs":
                        for kc in range(4):
                            nc.sync.dma_start(t[:, kc], src[:, kc])
                    else:
                        nc.sync.dma_start(t[:], src)
                    sb[nm] = t
            ones_col = cpool.tile([128, 1], BF16, tag="ones_col")
            nc.vector.memset(ones_col[:], 1.0)
            ones1k = cpool.tile([1, SLAB], BF16, tag="ones1k")
            nc.vector.memset(ones1k[:], 1.0)

            # DRAM bounce buffers for collectives: fused [K(+ones) ; V^T]
            KSZ = (CQ + 1) * SLAB
            VSZ = SLAB * CD
            kv_in, kv_out = {}, {}
            for m in MODS:
                kv_in[m] = dpool.tile([KSZ + VSZ], BF16, tag=f"kvi_{m}", name=f"kvi_{m}")
                kv_out[m] = dpool.tile([4, KSZ + VSZ], BF16, tag=f"kvo_{m}", name=f"kvo_{m}")

            conv_sb, convb_sb, qk_sb, vt_sb = {}, {}, {}, {}

            ppc_cm = tc.tile_pool(name="ppc", bufs=4, space="PSUM")
            ppc = ppc_cm.__enter__()

            # ---- per-modality: conv -> bn+prelu -> q/k/v projections ----
            for m in MODS:
                load_const(m, ["xs", "bna", "bnb", "alpha", "qkw", "qkb",
                               "vw", "gvb", "gam"])
                xs = sb[f"xs_{m}"]
                conv_sb[m] = bpool.tile([128, 2, SLAB], BF16, tag=f"conv_{m}", name=f"conv_{m}")
                convb_sb[m] = bpool.tile([128, 2, SLAB], BF16, tag=f"convb_{m}", name=f"convb_{m}")
                qk_sb[m] = bpool.tile([64, SLAB], BF16, tag=f"qk_{m}", name=f"qk_{m}")
                vt_sb[m] = bpool.tile([128, 8, CD], BF16, tag=f"vt_{m}", name=f"vt_{m}")

                pcv = [[None, None], [None, None]]
                for mc in range(2):
                    for n2 in range(2):
                        pcv[mc][n2] = ppc.tile([128, 512], F32, tag="pcv", name=f"pcv_{mc}_{n2}")
                for tap in range(9):
                    dy, dx = tap // 3, tap % 3
                    cwt = cwpool.tile([128, 4, CD], BF16, tag="cwt")
                    nc.sync.dma_start(
                        cwt[:], D[f"cw_{m}"][tap].rearrange("k p f -> p k f", p=128))
                    for kc in range(4):
                        for mc in range(2):
                            for n2 in range(2):
                                nc.tensor.matmul(
                                    pcv[mc][n2][:],
                                    cwt[:, kc, 128 * mc:128 * mc + 128],
                                    xs[:, kc, dy + 8 * n2: dy + 8 * n2 + 8,
                                       dx:dx + W],
                                    start=(tap == 0 and kc == 0),
                                    stop=(tap == 8 and kc == 3),
                                )
                for mc in range(2):
                    for n2 in range(2):
                        nc.scalar.activation(
                            conv_sb[m][:, mc, 512 * n2:512 * n2 + 512],
                            pcv[mc][n2][:], AF.Prelu,
                            bias=sb[f"bnb_{m}"][:, mc:mc + 1],
                            scale=sb[f"bna_{m}"][:, mc:mc + 1],
                            alpha=sb[f"alpha_{m}"][:, 0:1],
                        )
                # conv + gamma*v_b (residual-with-v-bias, exact through softmax)
                for mc in range(2):
                    nc.vector.tensor_scalar_add(
                        convb_sb[m][:, mc, :], conv_sb[m][:, mc, :],
                        sb[f"gvb_{m}"][:, mc:mc + 1])

                # q/k projections (64 = [q;k] channels)
                for n2 in range(2):
                    ps = ppc.tile([64, 512], F32, tag="pcv", name="ps_qk")
                    for kc in range(2):
                        nc.tensor.matmul(
                            ps[:], sb[f"qkw_{m}"][:, kc, :],
                            conv_sb[m][:, kc, 512 * n2:512 * n2 + 512],
                            start=(kc == 0), stop=(kc == 1))
                    nc.scalar.activation(
                        qk_sb[m][:, 512 * n2:512 * n2 + 512], ps[:],
                        AF.Identity, bias=sb[f"qkb_{m}"][:, 0:1])
                # ship K (+ ones row) into the fused collective buffer
                nc.sync.dma_start(
                    kv_in[m][0:CQ * SLAB].rearrange("(c u) -> c u", c=CQ),
                    qk_sb[m][32:64, :])
                nc.sync.dma_start(
                    kv_in[m][CQ * SLAB:KSZ].rearrange("(o u) -> o u", o=1),
                    ones1k[:])

                # V^T projection ([pix, c] layout; x8 scaled into fp8)
                for pc in range(8):
                    ps = ppc.tile([128, CD], F32, tag="pcv", name="ps_v")
                    for kc in range(2):
                        nc.tensor.matmul(
                            ps[:],
                            conv_sb[m][:, kc, 128 * pc:128 * pc + 128],
                            sb[f"vw_{m}"][:, kc, :],
                            start=(kc == 0), stop=(kc == 1))
                    nc.scalar.activation(vt_sb[m][:, pc, :], ps[:], AF.Copy)
                nc.sync.dma_start(
                    kv_in[m][KSZ:].rearrange("(pc p c) -> p pc c", p=128, c=CD),
                    vt_sb[m][:])
                nc.gpsimd.collective_compute(
                    "AllGather", ALU.bypass, replica_groups=RG,
                    ins=[kv_in[m].opt()], outs=[kv_out[m].opt()])

            for m in MODS:
                load_const(m, ["upw", "upb"])
            negI = cpool.tile([128, 128], F32, tag="negI")
            nc.sync.dma_start(negI[:], D["negI"])

            # ---- attention setup: per direction (qm, km) ----
            DIRS = (("dsm", "rgb"), ("rgb", "dsm"))
            KS2, QS2, K4, q4, vfull, mstack = {}, {}, {}, {}, {}, {}

            def setup_dir(di):
                qm, km = DIRS[di]
                # KS2: rows 0-31 K, row 32 ones; copy at rows 64-96 (2-way
                # PE-array packing of the S matmuls). Rows 33-63/97-127 unused.
                KS2[di] = prpool.tile([128, N], BF16, tag="KS2", name=f"KS2_{di}")
                for r0 in (0, 64):
                    nc.sync.dma_start(
                        KS2[di][r0:r0 + 33, :].rearrange("c (g u) -> c g u", g=4),
                        kv_out[km][:, 0:KSZ].rearrange(
                            "g (c u) -> c g u", c=CQ + 1))
                # K4/q4 for the row-max pass: 4 slab groups stacked at 32g
                K4[di] = prpool.tile([128, SLAB], BF16, tag="K4", name=f"K4_{di}")
                for g in range(4):
                    nc.sync.dma_start(
                        K4[di][32 * g:32 * g + 32, :],
                        kv_out[km][g, 0:CQ * SLAB].rearrange(
                            "(c u) -> c u", c=CQ))
                q4[di] = prpool.tile([128, SLAB], BF16, tag="q4", name=f"q4_{di}")
                for g in range(4):
                    nc.vector.tensor_copy(q4[di][32 * g:32 * g + 32, :],
                                          qk_sb[qm][0:32, :])
                QS2[di] = prpool.tile([128, SLAB], BF16, tag="QS2", name=f"QS2_{di}")
                for r0 in (0, 64):
                    nc.vector.tensor_copy(QS2[di][r0:r0 + 32, :], qk_sb[qm][0:32, :])
                mstack[di] = epool.tile([128, 8], F32, tag="mstack", bufs=2,
                                        name=f"mstack_{di}")

            def load_v(di):
                qm, km = DIRS[di]
                vfull[di] = prpool.tile([128, NKT, CD], BF16, tag="vfull",
                                        name=f"vfull_{di}")
                for t in range(NKT):
                    base = KSZ + (128 * (t % 8)) * CD
                    nc.sync.dma_start(
                        vfull[di][:, t, :],
                        kv_out[km][t // 8, base:base + 128 * CD].rearrange(
                            "(p c) -> p c", p=128))

            # ---- pass A (row maxes) ----
            dbg_passA = {}
            def passA_rounds(di, ics, ppa):
                engines = [nc.vector]
                mts = {}
                for ic in ics:
                    mts[ic] = epool.tile([128, 4], F32, tag="mt", bufs=4,
                                         name=f"mt_{di}_{ic}")
                    if dbg and di == 0 and ic == 0:
                        dbg_passA["mt00"] = mts[ic]
                for j in range(4):
                    g = j
                    for ei, ic in enumerate(ics):
                        psA = ppa.tile([128, 256], F32, tag="psl" if ppa.name == "ppl" else "psA", name="psA")
                        nc.tensor.matmul(
                            psA[:],
                            q4[di][32 * g:32 * g + 32, 128 * ic:128 * ic + 128],
                            K4[di][32 * g:32 * g + 32, 0:SLAB:4],
                            start=True, stop=True,
                            tile_position=(32 * g, 0))
                        engines[ei % len(engines)].reduce_max(
                            mts[ic][:, j:j + 1], psA[:], axis=AX.X)
                for ei, ic in enumerate(ics):
                    engines[ei % len(engines)].reduce_max(
                        mstack[di][:, ic:ic + 1], mts[ic][:, 0:4], axis=AX.X)

            def emit_mhalf(di, half, ppa):
                # transpose mstack[:, 4h:4h+4] -> [4,128], negate, write -m rows
                psT = ppa.tile([128, 512], F32, tag="psl" if ppa.name == "ppl" else "psA", name="psT")
                nc.tensor.transpose(psT[0:4, 0:128],
                                    mstack[di][:, 4 * half:4 * half + 4], negI[:])
                mneg0 = epool.tile([4, 128], F32, tag="mneg0", bufs=4, name="mneg0")
                nc.vector.tensor_scalar_mul(mneg0[:], psT[0:4, 0:128], -1.0)
                mneg = epool.tile([4, 128], BF16, tag="mneg", bufs=4, name="mneg")
                nc.vector.tensor_scalar_add(mneg[:], mneg0[:], -MARGIN)
                for r in (32, 96):
                    nc.sync.dma_start(
                        QS2[di][r:r + 1, 512 * half:512 * half + 512],
                        mneg[:])

            # dir-0 loads first (dir-1 loads wait on the second AllGather
            # and would block the DMA queue; they are emitted later).
            setup_dir(0)
            load_v(0)

            # pass A for direction 0 fully (own psum pool window)
            with tc.tile_pool(name="ppa1", bufs=4, space="PSUM") as ppa1:
                passA_rounds(0, [0, 1], ppa1)
                passA_rounds(0, [2, 3], ppa1)
                emit_mhalf(0, 0, ppa1)
                passA_rounds(0, [4, 5], ppa1)
                passA_rounds(0, [6, 7], ppa1)
                emit_mhalf(0, 1, ppa1)
            ppc_cm.__exit__(None, None, None)

            # ---- flash phases ----
            ppf_cm = tc.tile_pool(name="ppf", bufs=1, space="PSUM")
            ppf = ppf_cm.__enter__()
            ppl_cm = tc.tile_pool(name="ppl", bufs=1, space="PSUM")
            ppl = ppl_cm.__enter__()

            o_sb = {}
            dbg_tiles = {}

            def flash_phase(di, ic2, inject=None):
                """One query chunk of 512: S (packed), exp, O accumulate."""
                qm, km = DIRS[di]
                psO = [ppf.tile([128, 512], F32, tag="psO", bufs=3,
                                name=f"psO_{mc}") for mc in range(2)]
                lacc = ptpool.tile([128, 2, 512], BF16, tag="lacc", bufs=2,
                                   name="lacc")
                for g in range(16):
                    psSt = ppf.tile([128, 1024], F32, tag="psSt", bufs=2,
                                    name="psSt")
                    for tt in range(2):
                        t = 2 * g + tt
                        r0 = 64 * (tt % 2)
                        nc.tensor.matmul(
                            psSt[:, 512 * tt:512 * tt + 512],
                            KS2[di][r0:r0 + 33, 128 * t:128 * t + 128],
                            QS2[di][r0:r0 + 33, 512 * ic2:512 * ic2 + 512],
                            start=True, stop=True,
                            tile_position=(r0, 0))
                    PT = ptpool.tile([128, 1024], BF16, tag="PT", name="PT")
                    nc.scalar.activation(PT[:], psSt[:], AF.Exp)
                    if g == 0:
                        nc.vector.tensor_copy(
                            lacc[:], PT[:].rearrange("p (t u) -> p t u", t=2))
                    else:
                        nc.vector.tensor_add(
                            lacc[:], lacc[:],
                            PT[:].rearrange("p (t u) -> p t u", t=2))
                    for tt in range(2):
                        t = 2 * g + tt
                        for mc in range(2):
                            nc.tensor.matmul(
                                psO[mc][:],
                                vfull[di][:, t, 128 * mc:128 * mc + 128],
                                PT[:, 512 * tt:512 * tt + 512],
                                start=(t == 0), stop=(t == 31))
                    if inject is not None:
                        inject(g)
                # row-sum l via PE (ones), then recg = gamma / l
                psl = ppl.tile([1, 512], F32, tag="psl", name="psl")
                for tt in range(2):
                    nc.tensor.matmul(psl[:], ones_col[:], lacc[:, tt, :],
                                     start=(tt == 0), stop=(tt == 1))
                recf = epool.tile([1, 512], F32, tag="recf", bufs=2, name="recf")
                nc.vector.reciprocal(recf[:], psl[:])
                recg = epool.tile([1, 512], BF16, tag="recg", bufs=2, name="recg")
                nc.vector.tensor_scalar_mul(recg[:], recf[:],
                                            sb[f"gam_{km}"][0:1, 0:1])
                # drain psO quickly (frees PSUM), then epilogue on DVE
                ocp = [epool.tile([128, 512], BF16, tag=f"ocp{mc}", bufs=2,
                                  name=f"ocp{mc}") for mc in range(2)]
                for mc in range(2):
                    nc.scalar.activation(ocp[mc][:], psO[mc][:], AF.Copy)
                if dbg and di == 0 and ic2 == 0:
                    dbg_tiles["lnl00"] = lnl
                    dbg_tiles["recg00"] = recg
                    dbg_tiles["ocp00"] = ocp[0]
                rb = epool.tile([128, 512], BF16, tag="rb", bufs=2, name="rb")
                nc.gpsimd.partition_broadcast(rb[:], recg[:])
                if di not in o_sb:
                    o_sb[di] = bpool.tile([128, 2, SLAB], BF16, tag=f"osb_{di}",
                                          name=f"osb_{di}")
                for mc in range(2):
                    t1 = epool.tile([128, 512], BF16, tag="t1", bufs=2, name="t1")
                    nc.vector.tensor_tensor(t1[:], ocp[mc][:], rb[:], op=ALU.mult)
                    nc.vector.tensor_tensor(
                        o_sb[di][:, mc, 512 * ic2:512 * ic2 + 512], t1[:],
                        convb_sb[km][:, mc, 512 * ic2:512 * ic2 + 512],
                        op=ALU.add)

            def up_chunk(di, oc, n2, ppu):
                qm, km = DIRS[di]
                psu = ppu.tile([128, 512], F32, tag="psl" if ppu.name == "ppl" else "psu", name="psu")
                for kc in range(2):
                    nc.tensor.matmul(
                        psu[:],
                        sb[f"upw_{km}"][:, kc, 128 * oc:128 * oc + 128],
                        o_sb[di][:, kc, 512 * n2:512 * n2 + 512],
                        start=(kc == 0), stop=(kc == 1))
                tb = epool.tile([128, 512], F32, tag="tb", bufs=2, name="tb")
                nc.scalar.activation(tb[:], psu[:], AF.Identity,
                                     bias=sb[f"upb_{km}"][:, oc:oc + 1])
                xsr = epool.tile([128, 512], F32, tag="xsr", bufs=2, name="xsr")
                nc.vector.tensor_copy(
                    xsr[:].rearrange("p (r w) -> p r w", r=8),
                    sb[f"xs_{km}"][:, oc, 1 + 8 * n2: 9 + 8 * n2, 1:1 + W])
                ob = epool.tile([128, 512], F32, tag="ob", bufs=2, name="ob")
                nc.gpsimd.tensor_tensor(ob[:], tb[:], xsr[:], op=ALU.add)
                nc.sync.dma_start(
                    OUT[km][128 * oc:128 * oc + 128,
                            512 * n2:512 * n2 + 512], ob[:])

            flash_phase(0, 0)
            flash_phase(0, 1)

            # up-projection for direction 0 fills the second AllGather wait
            for n2 in range(2):
                for oc in range(4):
                    up_chunk(0, oc, n2, ppl)

            # dir-1 loads + row-max pass (gated on the dsm AllGather)
            setup_dir(1)
            load_v(1)
            for pair in ([0, 1], [2, 3], [4, 5], [6, 7]):
                passA_rounds(1, pair, ppl)
                if pair == [2, 3]:
                    emit_mhalf(1, 0, ppl)
            emit_mhalf(1, 1, ppl)

            up_steps = [(1, oc, 0) for oc in range(4)]

            def inject_up(g):
                if g in (2, 6, 10, 14) and up_steps:
                    di, oc, n2 = up_steps.pop(0)
                    up_chunk(di, oc, n2, ppl)

            flash_phase(1, 0)
            flash_phase(1, 1, inject=inject_up)
            for oc in range(4):
                up_chunk(1, oc, 1, ppl)
            ppl_cm.__exit__(None, None, None)
            ppf_cm.__exit__(None, None, None)
            if dbg:
                nc.sync.dma_start(DBG["dbg_conv_rgb"], conv_sb["rgb"][:])
                nc.sync.dma_start(DBG["dbg_qk_dsm"], qk_sb["dsm"][:])
                nc.sync.dma_start(DBG["dbg_mstack0"], mstack[0][:])
                nc.sync.dma_start(DBG["dbg_QS2_0"], QS2[0][:])
                nc.sync.dma_start(DBG["dbg_KS2_0"], KS2[0][:])
                nc.sync.dma_start(DBG["dbg_vfull0"], vfull[0][:])
                nc.sync.dma_start(DBG["dbg_lnl00"], dbg_tiles["lnl00"][:])
                nc.sync.dma_start(DBG["dbg_recg00"], dbg_tiles["recg00"][:])
                nc.sync.dma_start(DBG["dbg_ocp00"], dbg_tiles["ocp00"][:])
                nc.sync.dma_start(DBG["dbg_osb0"], o_sb[0][:])
                nc.sync.dma_start(DBG["dbg_K4_0"], K4[0][:])
                nc.sync.dma_start(DBG["dbg_q4_0"], q4[0][:])
                nc.sync.dma_start(DBG["dbg_mt00"], dbg_passA["mt00"][:])

    nc.compile()
    return nc


@functools.lru_cache(maxsize=2)
def _program(dbg=False):
    return _build(dbg)


def _prep_shared(inputs):
    bf16 = ml_dtypes.bfloat16
    W_ = {}
    for m in MODS:
        cw = np.asarray(inputs[f"conv_w_{m}"], np.float32)       # [CD,CIN,3,3]
        W_[f"cw_{m}"] = np.ascontiguousarray(
            cw.transpose(1, 2, 3, 0).reshape(4, 128, 3, 3, CD)
              .transpose(2, 3, 0, 1, 4).reshape(9, 4, 128, CD)).astype(bf16)
        g = np.asarray(inputs[f"bn_g_{m}"], np.float64)
        bb = np.asarray(inputs[f"bn_b_{m}"], np.float64)
        mu = np.asarray(inputs[f"bn_m_{m}"], np.float64)
        v = np.asarray(inputs[f"bn_v_{m}"], np.float64)
        cb = np.asarray(inputs[f"conv_b_{m}"], np.float64)
        scale = (g / np.sqrt(v + 1e-5))
        shift = bb - mu * scale + cb * scale     # fold conv bias into BN shift
        W_[f"bna_{m}"] = np.ascontiguousarray(
            scale.astype(np.float32).reshape(2, 128).T)
        W_[f"bnb_{m}"] = np.ascontiguousarray(
            shift.astype(np.float32).reshape(2, 128).T)
        W_[f"alpha_{m}"] = np.full((128, 1),
                                   np.float32(inputs[f"prelu_{m}"]), np.float32)
        gamma = float(np.float32(inputs[f"gamma_{m}"]))
        W_[f"gam_{m}"] = np.full((1, 1), gamma / VSCALE, np.float32)
        qk = np.concatenate([np.asarray(inputs[f"q_w_{m}"], np.float32),
                             np.asarray(inputs[f"k_w_{m}"], np.float32)], 0)
        W_[f"qkw_{m}"] = np.ascontiguousarray(
            qk.T.reshape(2, 128, 64)).astype(bf16)
        W_[f"qkb_{m}"] = np.concatenate(
            [np.asarray(inputs[f"q_b_{m}"], np.float32),
             np.asarray(inputs[f"k_b_{m}"], np.float32)], 0).reshape(64, 1)
        W_[f"vw_{m}"] = np.ascontiguousarray(
            np.asarray(inputs[f"v_w_{m}"], np.float32).T.reshape(2, 128, CD)
        ).astype(bf16)
        W_[f"upw_{m}"] = np.ascontiguousarray(
            np.asarray(inputs[f"up_w_{m}"], np.float32).T.reshape(2, 128, CIN)
        ).astype(bf16)
        W_[f"upb_{m}"] = np.ascontiguousarray(
            np.asarray(inputs[f"up_b_{m}"], np.float32).reshape(4, 128).T)
        gvb = (np.float32(inputs[f"gamma_{m}"])
               * np.asarray(inputs[f"v_b_{m}"], np.float32))
        W_[f"gvb_{m}"] = np.ascontiguousarray(gvb.reshape(2, 128).T)
    W_["negI"] = -np.eye(128, dtype=np.float32)
    return W_


def _slab(x_b, s):
    xp = np.zeros((CIN, HR, WP), np.float32)
    r0 = SLAB_ROWS * s - 1
    lo, hi = max(r0, 0), min(r0 + HR, H)
    xp[:, lo - r0:hi - r0, 1:1 + W] = x_b[:, lo:hi, :]
    return np.ascontiguousarray(
        xp.reshape(4, 128, HR, WP).transpose(1, 0, 2, 3)).astype(
            ml_dtypes.bfloat16)


def _make_in_maps(inputs):
    W_ = _prep_shared(inputs)
    xin = {m: np.asarray(inputs[f"input_{m}"], np.float32) for m in MODS}
    in_maps = []
    for cid in range(N_CORES):
        b, s = cid // 4, cid % 4
        im = dict(W_)
        for m in MODS:
            im[f"xs_{m}"] = _slab(xin[m][b], s)
        in_maps.append(im)
    return in_maps


def kernel(**inputs):
    nc = _program()
    in_maps = _make_in_maps(inputs)
    res = run_bass_kernel_spmd(nc, in_maps, core_ids=list(range(N_CORES)))
    out = {m: np.zeros((B, CIN, H, W), np.float32) for m in MODS}
    for cid in range(N_CORES):
        b, s = cid // 4, cid % 4
        for m in MODS:
            out[m][b, :, SLAB_ROWS * s:SLAB_ROWS * (s + 1), :] = (
                res.results[cid][f"out_{m}"].reshape(CIN, SLAB_ROWS, W))
    return (out["rgb"], out["dsm"])


# revision 20
# speedup vs baseline: 1.0497x; 1.0497x over previous
"""Trainium2 Bass kernel for nn_CrossSemanticAttentionModule0 (cross-modal attention).

Sharding: 8 cores = (batch b in {0,1}) x (query/pixel slab s in {0..3}; 16 H-rows
= 1024 pixels each). Each core: conv+BN+PReLU for its slab (with halo), q/k/v
projections, AllGathers K (bf16, with a ones row for the softmax bias trick) and
V^T (fp8 e4m3, x8 scaled) across its 4-core batch group, then two-pass softmax
cross-attention for its query rows over the full key axis, up-projections +
residuals. All matmuls in bf16 (fp32 PSUM accumulate).
"""

import numpy as np
import functools

import ml_dtypes

import concourse.bass as bass
import concourse.mybir as mybir
import concourse.tile as tile
import concourse.bacc as bacc
from concourse.bass_utils import run_bass_kernel_spmd

B, CIN, H, W = 2, 512, 64, 64
CD, CQ = 256, 32
N = H * W                 # 4096 pixels
SLAB_ROWS = 16            # H rows per core
SLAB = SLAB_ROWS * W      # 1024 pixels per core
HR = SLAB_ROWS + 2        # halo rows
WP = W + 2                # padded width
N_CORES = 8
MODS = ("rgb", "dsm")
F32 = mybir.dt.float32
BF16 = mybir.dt.bfloat16
FP8 = mybir.dt.float8e4
AF = mybir.ActivationFunctionType
ALU = mybir.AluOpType
AX = mybir.AxisListType
RG = [[0, 1, 2, 3], [4, 5, 6, 7]]
NKT = N // 128            # 32 key tiles of 128
VSCALE = 4.0              # V carried x4 in fp8 on the wire
MARGIN = 72.0             # softmax max-bound margin over stride-4 subsample


def _build(dbg=False):
    nc = bacc.Bacc("TRN2", target_bir_lowering=False, debug=False,
                   num_devices=N_CORES)
    DBG = {}
    def dbg_out(name, shape, dt):
        if dbg:
            DBG[name] = nc.dram_tensor(name, shape, dt,
                                       kind="ExternalOutput").ap()
    dbg_out("dbg_conv_rgb", [128, 2, SLAB], BF16)
    dbg_out("dbg_qk_dsm", [64, SLAB], BF16)
    dbg_out("dbg_mstack0", [128, 8], F32)
    dbg_out("dbg_QS2_0", [128, SLAB], BF16)
    dbg_out("dbg_KS2_0", [128, N], BF16)
    dbg_out("dbg_vfull0", [128, NKT, CD], BF16)
    dbg_out("dbg_lnl00", [1, 512], F32)
    dbg_out("dbg_recg00", [1, 512], BF16)
    dbg_out("dbg_ocp00", [128, 512], BF16)
    dbg_out("dbg_osb0", [128, 2, SLAB], BF16)
    dbg_out("dbg_K4_0", [128, SLAB], BF16)
    dbg_out("dbg_q4_0", [128, SLAB], BF16)
    dbg_out("dbg_mt00", [128, 4], F32)

    D = {}
    def din(name, shape, dt=F32):
        D[name] = nc.dram_tensor(name, shape, dt, kind="ExternalInput").ap()
    for m in MODS:
        din(f"xs_{m}", [128, 4, HR, WP], BF16)
        din(f"cw_{m}", [9, 4, 128, CD], BF16)
        din(f"bna_{m}", [128, 2])
        din(f"bnb_{m}", [128, 2])
        din(f"alpha_{m}", [128, 1])
        din(f"qkw_{m}", [2, 128, 64], BF16)
        din(f"qkb_{m}", [64, 1])
        din(f"vw_{m}", [2, 128, CD], BF16)
        din(f"upw_{m}", [2, 128, CIN], BF16)
        din(f"upb_{m}", [128, 4])
        din(f"gvb_{m}", [128, 2])
        din(f"gam_{m}", [1, 1])
    din("negI", [128, 128])
    OUT = {m: nc.dram_tensor(f"out_{m}", [CIN, SLAB], F32,
                             kind="ExternalOutput").ap() for m in MODS}

    with tile.TileContext(nc) as tc:
        with (
            tc.tile_pool(name="const", bufs=1) as cpool,
            tc.tile_pool(name="cw", bufs=3) as cwpool,
            tc.tile_pool(name="big", bufs=1) as bpool,
            tc.tile_pool(name="pair", bufs=2) as prpool,
            tc.tile_pool(name="pt", bufs=3) as ptpool,
            tc.tile_pool(name="eps", bufs=2) as epool,
            tc.tile_pool(name="dram", bufs=1, space="DRAM") as dpool,
        ):
            # ---- constants / weights to SBUF (loaded just-in-time) ----
            sb = {}
            def load_const(m, names):
                specs = {
                    "xs": ([128, 4, HR, WP], BF16), "bna": ([128, 2], F32),
                    "bnb": ([128, 2], F32), "alpha": ([128, 1], F32),
                    "qkw": ([128, 2, 64], BF16), "qkb": ([64, 1], F32),
                    "vw": ([128, 2, CD], BF16), "upw": ([128, 2, CIN], BF16),
                    "upb": ([128, 4], F32), "gvb": ([128, 2], F32),
                    "gam": ([1, 1], F32),
                }
                for base in names:
                    nm = f"{base}_{m}"
                    shp, dt = specs[base]
                    t = cpool.tile(shp, dt, tag=nm, name=nm)
                    src = D[nm]
                    if base in ("qkw", "vw", "upw"):
                        src = src.rearrange("k p f -> p k f", p=128)
                    if base == "xs":
                        for kc in range(4):
                            nc.sync.dma_start(t[:, kc], src[:, kc])
                    else:
                        nc.sync.dma_start(t[:], src)
                    sb[nm] = t
            ones_col = cpool.tile([128, 1], BF16, tag="ones_col")
            nc.vector.memset(ones_col[:], 1.0)
            ones1k = cpool.tile([1, SLAB], BF16, tag="ones1k")
            nc.vector.memset(ones1k[:], 1.0)

            # DRAM bounce buffers for collectives: fused [K(+ones) bf16 ; V^T fp8]
            KSZ = (CQ + 1) * SLAB          # K elems (bf16; 2 bytes each)
            VSZ = SLAB * CD                # V elems (fp8; 1 byte each)
            kv_in, kv_out = {}, {}
            for m in MODS:
                kv_in[m] = dpool.tile([2 * KSZ + VSZ], FP8, tag=f"kvi_{m}", name=f"kvi_{m}")
                kv_out[m] = dpool.tile([4, 2 * KSZ + VSZ], FP8, tag=f"kvo_{m}", name=f"kvo_{m}")

            conv_sb, convb_sb, qk_sb, vt_sb = {}, {}, {}, {}

            ppc_cm = tc.tile_pool(name="ppc", bufs=4, space="PSUM")
            ppc = ppc_cm.__enter__()

            # ---- per-modality: conv -> bn+prelu -> q/k/v projections ----
            for m in MODS:
                load_const(m, ["xs", "bna", "bnb", "alpha", "qkw", "qkb",
                               "vw", "gvb", "gam"])
                xs = sb[f"xs_{m}"]
                conv_sb[m] = bpool.tile([128, 2, SLAB], BF16, tag=f"conv_{m}", name=f"conv_{m}")
                convb_sb[m] = bpool.tile([128, 2, SLAB], BF16, tag=f"convb_{m}", name=f"convb_{m}")
                qk_sb[m] = bpool.tile([64, SLAB], BF16, tag=f"qk_{m}", name=f"qk_{m}")
                vt_sb[m] = bpool.tile([128, 8, CD], FP8, tag=f"vt_{m}", name=f"vt_{m}")

                pcv = [[None, None], [None, None]]
                for mc in range(2):
                    for n2 in range(2):
                        pcv[mc][n2] = ppc.tile([128, 512], F32, tag="pcv", name=f"pcv_{mc}_{n2}")
                for tap in range(9):
                    dy, dx = tap // 3, tap % 3
                    cwt = cwpool.tile([128, 4, CD], BF16, tag="cwt")
                    nc.sync.dma_start(
                        cwt[:], D[f"cw_{m}"][tap].rearrange("k p f -> p k f", p=128))
                    for kc in range(4):
                        for mc in range(2):
                            for n2 in range(2):
                                nc.tensor.matmul(
                                    pcv[mc][n2][:],
                                    cwt[:, kc, 128 * mc:128 * mc + 128],
                                    xs[:, kc, dy + 8 * n2: dy + 8 * n2 + 8,
                                       dx:dx + W],
                                    start=(tap == 0 and kc == 0),
                                    stop=(tap == 8 and kc == 3),
                                )
                for mc in range(2):
                    for n2 in range(2):
                        nc.scalar.activation(
                            conv_sb[m][:, mc, 512 * n2:512 * n2 + 512],
                            pcv[mc][n2][:], AF.Prelu,
                            bias=sb[f"bnb_{m}"][:, mc:mc + 1],
                            scale=sb[f"bna_{m}"][:, mc:mc + 1],
                            alpha=sb[f"alpha_{m}"][:, 0:1],
                        )
                # conv + gamma*v_b (residual-with-v-bias, exact through softmax)
                for mc in range(2):
                    nc.vector.tensor_scalar_add(
                        convb_sb[m][:, mc, :], conv_sb[m][:, mc, :],
                        sb[f"gvb_{m}"][:, mc:mc + 1])

                # q/k projections (64 = [q;k] channels)
                for n2 in range(2):
                    ps = ppc.tile([64, 512], F32, tag="pcv", name="ps_qk")
                    for kc in range(2):
                        nc.tensor.matmul(
                            ps[:], sb[f"qkw_{m}"][:, kc, :],
                            conv_sb[m][:, kc, 512 * n2:512 * n2 + 512],
                            start=(kc == 0), stop=(kc == 1))
                    nc.scalar.activation(
                        qk_sb[m][:, 512 * n2:512 * n2 + 512], ps[:],
                        AF.Identity, bias=sb[f"qkb_{m}"][:, 0:1])
                # ship K (+ ones row) into the fused collective buffer
                nc.sync.dma_start(
                    kv_in[m][0:2 * CQ * SLAB].bitcast(BF16).rearrange(
                        "(c u) -> c u", c=CQ),
                    qk_sb[m][32:64, :])
                nc.sync.dma_start(
                    kv_in[m][2 * CQ * SLAB:2 * KSZ].bitcast(BF16).rearrange(
                        "(o u) -> o u", o=1),
                    ones1k[:])

                # V^T projection ([pix, c] layout; x8 scaled into fp8)
                for pc in range(8):
                    ps = ppc.tile([128, CD], F32, tag="pcv", name="ps_v")
                    for kc in range(2):
                        nc.tensor.matmul(
                            ps[:],
                            conv_sb[m][:, kc, 128 * pc:128 * pc + 128],
                            sb[f"vw_{m}"][:, kc, :],
                            start=(kc == 0), stop=(kc == 1))
                    nc.scalar.activation(vt_sb[m][:, pc, :], ps[:],
                                         AF.Copy, scale=VSCALE)
                nc.sync.dma_start(
                    kv_in[m][2 * KSZ:].rearrange("(pc p c) -> p pc c", p=128, c=CD),
                    vt_sb[m][:])
                nc.gpsimd.collective_compute(
                    "AllGather", ALU.bypass, replica_groups=RG,
                    ins=[kv_in[m].opt()], outs=[kv_out[m].opt()])

            for m in MODS:
                load_const(m, ["upw", "upb"])
            negI = cpool.tile([128, 128], F32, tag="negI")
            nc.sync.dma_start(negI[:], D["negI"])

            # ---- attention setup: per direction (qm, km) ----
            DIRS = (("dsm", "rgb"), ("rgb", "dsm"))
            KS2, QS2, K4, q4, vfull, mstack = {}, {}, {}, {}, {}, {}

            def setup_dir(di):
                qm, km = DIRS[di]
                # KS2: rows 0-31 K, row 32 ones; copy at rows 64-96 (2-way
                # PE-array packing of the S matmuls). Rows 33-63/97-127 unused.
                KS2[di] = prpool.tile([128, N], BF16, tag="KS2", name=f"KS2_{di}")
                for r0 in (0, 64):
                    nc.sync.dma_start(
                        KS2[di][r0:r0 + 33, :].rearrange("c (g u) -> c g u", g=4),
                        kv_out[km][:, 0:2 * KSZ].bitcast(BF16).rearrange(
                            "g (c u) -> c g u", c=CQ + 1))
                # K4/q4 for the row-max pass: 4 slab groups stacked at 32g
                K4[di] = prpool.tile([128, SLAB], BF16, tag="K4", name=f"K4_{di}")
                for g in range(4):
                    nc.sync.dma_start(
                        K4[di][32 * g:32 * g + 32, :],
                        kv_out[km][g, 0:2 * CQ * SLAB].bitcast(BF16).rearrange(
                            "(c u) -> c u", c=CQ))
                q4[di] = prpool.tile([128, SLAB], BF16, tag="q4", name=f"q4_{di}")
                for g in range(4):
                    nc.vector.tensor_copy(q4[di][32 * g:32 * g + 32, :],
                                          qk_sb[qm][0:32, :])
                QS2[di] = prpool.tile([128, SLAB], BF16, tag="QS2", name=f"QS2_{di}")
                for r0 in (0, 64):
                    nc.vector.tensor_copy(QS2[di][r0:r0 + 32, :], qk_sb[qm][0:32, :])
                mstack[di] = epool.tile([128, 8], F32, tag="mstack", bufs=2,
                                        name=f"mstack_{di}")

            def load_v(di):
                qm, km = DIRS[di]
                vf8 = prpool.tile([128, NKT, CD], FP8, tag="vf8", bufs=1,
                                  name=f"vf8_{di}")
                for t in range(NKT):
                    base = 2 * KSZ + (128 * (t % 8)) * CD
                    nc.sync.dma_start(
                        vf8[:, t, :],
                        kv_out[km][t // 8, base:base + 128 * CD].rearrange(
                            "(p c) -> p c", p=128))
                vfull[di] = prpool.tile([128, NKT, CD], BF16, tag="vfull",
                                        name=f"vfull_{di}")
                for i in range(8):
                    eng = nc.vector if i % 2 == 0 else nc.gpsimd
                    eng.tensor_copy(vfull[di][:, 4 * i:4 * i + 4, :],
                                    vf8[:, 4 * i:4 * i + 4, :])

            # ---- pass A (row maxes) ----
            dbg_passA = {}
            def passA_rounds(di, ics, ppa):
                engines = [nc.vector]
                mts = {}
                for ic in ics:
                    mts[ic] = epool.tile([128, 4], F32, tag="mt", bufs=4,
                                         name=f"mt_{di}_{ic}")
                    if dbg and di == 0 and ic == 0:
                        dbg_passA["mt00"] = mts[ic]
                for j in range(4):
                    g = j
                    for ei, ic in enumerate(ics):
                        psA = ppa.tile([128, 256], F32, tag="psl" if ppa.name == "ppl" else "psA", name="psA")
                        nc.tensor.matmul(
                            psA[:],
                            q4[di][32 * g:32 * g + 32, 128 * ic:128 * ic + 128],
                            K4[di][32 * g:32 * g + 32, 0:SLAB:4],
                            start=True, stop=True,
                            tile_position=(32 * g, 0))
                        engines[ei % len(engines)].reduce_max(
                            mts[ic][:, j:j + 1], psA[:], axis=AX.X)
                for ei, ic in enumerate(ics):
                    engines[ei % len(engines)].reduce_max(
                        mstack[di][:, ic:ic + 1], mts[ic][:, 0:4], axis=AX.X)

            def emit_mhalf(di, half, ppa):
                # transpose mstack[:, 4h:4h+4] -> [4,128], negate, write -m rows
                psT = ppa.tile([128, 512], F32, tag="psl" if ppa.name == "ppl" else "psA", name="psT")
                nc.tensor.transpose(psT[0:4, 0:128],
                                    mstack[di][:, 4 * half:4 * half + 4], negI[:])
                mneg0 = epool.tile([4, 128], F32, tag="mneg0", bufs=4, name="mneg0")
                nc.vector.tensor_scalar_mul(mneg0[:], psT[0:4, 0:128], -1.0)
                mneg = epool.tile([4, 128], BF16, tag="mneg", bufs=4, name="mneg")
                nc.vector.tensor_scalar_add(mneg[:], mneg0[:], -MARGIN)
                for r in (32, 96):
                    nc.sync.dma_start(
                        QS2[di][r:r + 1, 512 * half:512 * half + 512],
                        mneg[:])

            # dir-0 loads first (dir-1 loads wait on the second AllGather
            # and would block the DMA queue; they are emitted later).
            setup_dir(0)
            load_v(0)

            # pass A for direction 0 fully (own psum pool window)
            with tc.tile_pool(name="ppa1", bufs=4, space="PSUM") as ppa1:
                passA_rounds(0, [0, 1], ppa1)
                passA_rounds(0, [2, 3], ppa1)
                emit_mhalf(0, 0, ppa1)
                passA_rounds(0, [4, 5], ppa1)
                passA_rounds(0, [6, 7], ppa1)
                emit_mhalf(0, 1, ppa1)
            ppc_cm.__exit__(None, None, None)

            # ---- flash phases ----
            ppf_cm = tc.tile_pool(name="ppf", bufs=1, space="PSUM")
            ppf = ppf_cm.__enter__()
            ppl_cm = tc.tile_pool(name="ppl", bufs=1, space="PSUM")
            ppl = ppl_cm.__enter__()

            o_sb = {}
            dbg_tiles = {}

            def flash_phase(di, ic2, inject=None):
                """One query chunk of 512: S (packed), exp, O accumulate."""
                qm, km = DIRS[di]
                psO = [ppf.tile([128, 512], F32, tag="psO", bufs=3,
                                name=f"psO_{mc}") for mc in range(2)]
                lacc = ptpool.tile([128, 2, 512], BF16, tag="lacc", bufs=2,
                                   name="lacc")
                for g in range(16):
                    psSt = ppf.tile([128, 1024], F32, tag="psSt", bufs=2,
                                    name="psSt")
                    for tt in range(2):
                        t = 2 * g + tt
                        r0 = 64 * (tt % 2)
                        nc.tensor.matmul(
                            psSt[:, 512 * tt:512 * tt + 512],
                            KS2[di][r0:r0 + 33, 128 * t:128 * t + 128],
                            QS2[di][r0:r0 + 33, 512 * ic2:512 * ic2 + 512],
                            start=True, stop=True,
                            tile_position=(r0, 0))
                    PT = ptpool.tile([128, 1024], BF16, tag="PT", name="PT")
                    nc.scalar.activation(PT[:], psSt[:], AF.Exp)
                    if g == 0:
                        nc.vector.tensor_copy(
                            lacc[:], PT[:].rearrange("p (t u) -> p t u", t=2))
                    else:
                        nc.vector.tensor_add(
                            lacc[:], lacc[:],
                            PT[:].rearrange("p (t u) -> p t u", t=2))
                    for tt in range(2):
                        t = 2 * g + tt
                        for mc in range(2):
                            nc.tensor.matmul(
                                psO[mc][:],
                                vfull[di][:, t, 128 * mc:128 * mc + 128],
                                PT[:, 512 * tt:512 * tt + 512],
                                start=(t == 0), stop=(t == 31))
                    if inject is not None:
                        inject(g)
                # row-sum l via PE (ones), then recg = gamma / l
                psl = ppl.tile([1, 512], F32, tag="psl", name="psl")
                for tt in range(2):
                    nc.tensor.matmul(psl[:], ones_col[:], lacc[:, tt, :],
                                     start=(tt == 0), stop=(tt == 1))
                recf = epool.tile([1, 512], F32, tag="recf", bufs=2, name="recf")
                nc.vector.reciprocal(recf[:], psl[:])
                recg = epool.tile([1, 512], BF16, tag="recg", bufs=2, name="recg")
                nc.vector.tensor_scalar_mul(recg[:], recf[:],
                                            sb[f"gam_{km}"][0:1, 0:1])
                # drain psO quickly (frees PSUM), then epilogue on DVE
                ocp = [epool.tile([128, 512], BF16, tag=f"ocp{mc}", bufs=2,
                                  name=f"ocp{mc}") for mc in range(2)]
                for mc in range(2):
                    nc.scalar.activation(ocp[mc][:], psO[mc][:], AF.Copy)
                if dbg and di == 0 and ic2 == 0:
                    dbg_tiles["lnl00"] = lnl
                    dbg_tiles["recg00"] = recg
                    dbg_tiles["ocp00"] = ocp[0]
                rb = epool.tile([128, 512], BF16, tag="rb", bufs=2, name="rb")
                nc.gpsimd.partition_broadcast(rb[:], recg[:])
                if di not in o_sb:
                    o_sb[di] = bpool.tile([128, 2, SLAB], BF16, tag=f"osb_{di}",
                                          name=f"osb_{di}")
                for mc in range(2):
                    t1 = epool.tile([128, 512], BF16, tag="t1", bufs=2, name="t1")
                    nc.vector.tensor_tensor(t1[:], ocp[mc][:], rb[:], op=ALU.mult)
                    nc.vector.tensor_tensor(
                        o_sb[di][:, mc, 512 * ic2:512 * ic2 + 512], t1[:],
                        convb_sb[km][:, mc, 512 * ic2:512 * ic2 + 512],
                        op=ALU.add)

            def up_chunk(di, oc, n2, ppu):
                qm, km = DIRS[di]
                psu = ppu.tile([128, 512], F32, tag="psl" if ppu.name == "ppl" else "psu", name="psu")
                for kc in range(2):
                    nc.tensor.matmul(
                        psu[:],
                        sb[f"upw_{km}"][:, kc, 128 * oc:128 * oc + 128],
                        o_sb[di][:, kc, 512 * n2:512 * n2 + 512],
                        start=(kc == 0), stop=(kc == 1))
                tb = epool.tile([128, 512], F32, tag="tb", bufs=2, name="tb")
                nc.scalar.activation(tb[:], psu[:], AF.Identity,
                                     bias=sb[f"upb_{km}"][:, oc:oc + 1])
                xsr = epool.tile([128, 512], F32, tag="xsr", bufs=2, name="xsr")
                nc.vector.tensor_copy(
                    xsr[:].rearrange("p (r w) -> p r w", r=8),
                    sb[f"xs_{km}"][:, oc, 1 + 8 * n2: 9 + 8 * n2, 1:1 + W])
                ob = epool.tile([128, 512], F32, tag="ob", bufs=2, name="ob")
                nc.gpsimd.tensor_tensor(ob[:], tb[:], xsr[:], op=ALU.add)
                nc.sync.dma_start(
                    OUT[km][128 * oc:128 * oc + 128,
                            512 * n2:512 * n2 + 512], ob[:])

            flash_phase(0, 0)
            flash_phase(0, 1)

            # up-projection for direction 0 fills the second AllGather wait
            for n2 in range(2):
                for oc in range(4):
                    up_chunk(0, oc, n2, ppl)

            # dir-1 loads + row-max pass (gated on the dsm AllGather)
            setup_dir(1)
            load_v(1)
            for pair in ([0, 1], [2, 3], [4, 5], [6, 7]):
                passA_rounds(1, pair, ppl)
                if pair == [2, 3]:
                    emit_mhalf(1, 0, ppl)
            emit_mhalf(1, 1, ppl)

            up_steps = [(1, oc, 0) for oc in range(4)]

            def inject_up(g):
                if g in (2, 6, 10, 14) and up_steps:
                    di, oc, n2 = up_steps.pop(0)
                    up_chunk(di, oc, n2, ppl)

            flash_phase(1, 0)
            flash_phase(1, 1, inject=inject_up)
            for oc in range(4):
                up_chunk(1, oc, 1, ppl)
            ppl_cm.__exit__(None, None, None)
            ppf_cm.__exit__(None, None, None)
            if dbg:
                nc.sync.dma_start(DBG["dbg_conv_rgb"], conv_sb["rgb"][:])
                nc.sync.dma_start(DBG["dbg_qk_dsm"], qk_sb["dsm"][:])
                nc.sync.dma_start(DBG["dbg_mstack0"], mstack[0][:])
                nc.sync.dma_start(DBG["dbg_QS2_0"], QS2[0][:])
                nc.sync.dma_start(DBG["dbg_KS2_0"], KS2[0][:])
                nc.sync.dma_start(DBG["dbg_vfull0"], vfull[0][:])
                nc.sync.dma_start(DBG["dbg_lnl00"], dbg_tiles["lnl00"][:])
                nc.sync.dma_start(DBG["dbg_recg00"], dbg_tiles["recg00"][:])
                nc.sync.dma_start(DBG["dbg_ocp00"], dbg_tiles["ocp00"][:])
                nc.sync.dma_start(DBG["dbg_osb0"], o_sb[0][:])
                nc.sync.dma_start(DBG["dbg_K4_0"], K4[0][:])
                nc.sync.dma_start(DBG["dbg_q4_0"], q4[0][:])
                nc.sync.dma_start(DBG["dbg_mt00"], dbg_passA["mt00"][:])

    nc.compile()
    return nc


@functools.lru_cache(maxsize=2)
def _program(dbg=False):
    return _build(dbg)


def _prep_shared(inputs):
    bf16 = ml_dtypes.bfloat16
    W_ = {}
    for m in MODS:
        cw = np.asarray(inputs[f"conv_w_{m}"], np.float32)       # [CD,CIN,3,3]
        W_[f"cw_{m}"] = np.ascontiguousarray(
            cw.transpose(1, 2, 3, 0).reshape(4, 128, 3, 3, CD)
              .transpose(2, 3, 0, 1, 4).reshape(9, 4, 128, CD)).astype(bf16)
        g = np.asarray(inputs[f"bn_g_{m}"], np.float64)
        bb = np.asarray(inputs[f"bn_b_{m}"], np.float64)
        mu = np.asarray(inputs[f"bn_m_{m}"], np.float64)
        v = np.asarray(inputs[f"bn_v_{m}"], np.float64)
        cb = np.asarray(inputs[f"conv_b_{m}"], np.float64)
        scale = (g / np.sqrt(v + 1e-5))
        shift = bb - mu * scale + cb * scale     # fold conv bias into BN shift
        W_[f"bna_{m}"] = np.ascontiguousarray(
            scale.astype(np.float32).reshape(2, 128).T)
        W_[f"bnb_{m}"] = np.ascontiguousarray(
            shift.astype(np.float32).reshape(2, 128).T)
        W_[f"alpha_{m}"] = np.full((128, 1),
                                   np.float32(inputs[f"prelu_{m}"]), np.float32)
        gamma = float(np.float32(inputs[f"gamma_{m}"]))
        W_[f"gam_{m}"] = np.full((1, 1), gamma / VSCALE, np.float32)
        qk = np.concatenate([np.asarray(inputs[f"q_w_{m}"], np.float32),
                             np.asarray(inputs[f"k_w_{m}"], np.float32)], 0)
        W_[f"qkw_{m}"] = np.ascontiguousarray(
            qk.T.reshape(2, 128, 64)).astype(bf16)
        W_[f"qkb_{m}"] = np.concatenate(
            [np.asarray(inputs[f"q_b_{m}"], np.float32),
             np.asarray(inputs[f"k_b_{m}"], np.float32)], 0).reshape(64, 1)
        W_[f"vw_{m}"] = np.ascontiguousarray(
            np.asarray(inputs[f"v_w_{m}"], np.float32).T.reshape(2, 128, CD)
        ).astype(bf16)
        W_[f"upw_{m}"] = np.ascontiguousarray(
            np.asarray(inputs[f"up_w_{m}"], np.float32).T.reshape(2, 128, CIN)
        ).astype(bf16)
        W_[f"upb_{m}"] = np.ascontiguousarray(
            np.asarray(inputs[f"up_b_{m}"], np.float32).reshape(4, 128).T)
        gvb = (np.float32(inputs[f"gamma_{m}"])
               * np.asarray(inputs[f"v_b_{m}"], np.float32))
        W_[f"gvb_{m}"] = np.ascontiguousarray(gvb.reshape(2, 128).T)
    W_["negI"] = -np.eye(128, dtype=np.float32)
    return W_


def _slab(x_b, s):
    xp = np.zeros((CIN, HR, WP), np.float32)
    r0 = SLAB_ROWS * s - 1
    lo, hi = max(r0, 0), min(r0 + HR, H)
    xp[:, lo - r0:hi - r0, 1:1 + W] = x_b[:, lo:hi, :]
    return np.ascontiguousarray(
        xp.reshape(4, 128, HR, WP).transpose(1, 0, 2, 3)).astype(
            ml_dtypes.bfloat16)


def _make_in_maps(inputs):
    W_ = _prep_shared(inputs)
    xin = {m: np.asarray(inputs[f"input_{m}"], np.float32) for m in MODS}
    in_maps = []
    for cid in range(N_CORES):
        b, s = cid // 4, cid % 4
        im = dict(W_)
        for m in MODS:
            im[f"xs_{m}"] = _slab(xin[m][b], s)
        in_maps.append(im)
    return in_maps


def kernel(**inputs):
    nc = _program()
    in_maps = _make_in_maps(inputs)
    res = run_bass_kernel_spmd(nc, in_maps, core_ids=list(range(N_CORES)))
    out = {m: np.zeros((B, CIN, H, W), np.float32) for m in MODS}
    for cid in range(N_CORES):
        b, s = cid // 4, cid % 4
        for m in MODS:
            out[m][b, :, SLAB_ROWS * s:SLAB_ROWS * (s + 1), :] = (
                res.results[cid][f"out_{m}"].reshape(CIN, SLAB_ROWS, W))
    return (out["rgb"], out["dsm"])
